# revision 39
# baseline (speedup 1.0000x reference)
"""TRN2 Bass kernel for nn_FP8LinearWrapper: y = x @ (w_fp8 * inv_scale).T + bias.

Strategy (8 NeuronCores, SPMD):
  - Data-parallel over the flattened token dim: x [4,2048,4096] -> [8192,4096],
    1024 rows per core. Weights/bias replicated to every core.
  - Per core: SINGLE-pass bf16 matmul. x arrives host-blocked to the
    transposed tile layout [MT, P(d), KT, P(m)] (pure layout prep, same
    class as the weight transpose/re-block below - no host arithmetic);
    the device rounds it to bf16 (ACT cast, the only arithmetic on x) into
    a resident SBUF operand. The fp8 weight is fed directly as the matmul
    moving operand (mixed bf16 x fp8e4 matmul, verified bit-exact on HW);
    accumulation in fp32 PSUM. Error is dominated by the bf16 rounding of
    x: ~1.7e-3 rel absmax vs the 2e-2 gate (the dual-pass hi+lo scheme
    this replaces was 2x the PE work for accuracy the gate does not need).
  - The fp8 weight bytes are jax float8_e4m3fn (max 448). TRN2's fp8e4 decode
    is IEEE e4m3 (max 240), so the host re-encodes each byte via a LUT to the
    e4m3 bits of (value/2) - exact for all normals - and the kernel folds the
    missing *2 into the output scale. w is passed pre-transposed/pre-blocked
    (weight layout prep, as for any serving stack).

Timing structure (measured ~472 us/core total; PE runs ONLY the 2048
[128k x 128m x 512o] matmuls at the 216 ns/MM N=512 bf16 streaming roofline
= 442 us/core and is ~99.7% busy within its span; x casts ride the idle ACT
engine, evictions the DVE; launch ~15 us, drain ~13 us):
  - Phase T: m-tile pairs; per pair the PE runs o-blocks 0..1 (128 matmuls)
    while the NEXT pair's x streams in (2MB DMA + 2 half-casts per m-tile).
    x-chains are emitted one pair BEHIND the matmuls; pair 0 runs as
    per-m-tile singles so the first matmul needs only m-tile 0's chunk.
  - Phase B: o-blocks 2..7 stream w fp8 from DRAM against the resident xt.
  - m-tile pairs share one 2-bank PSUM tile (2 x 32 accumulating matmuls)
    with a single fused (psum * 2*inv_scale) + bias DVE eviction covering
    both banks; with no PE-transpose PSUM pool needed, the matmul pool gets
    4 bufs (all 8 banks) so evictions never gate group starts.
Rejected alternatives (measured on HW):
  - On-device PE transposes (prior version, 490 us): 256 transposes pace at
    the ~130 ns LDWEIGHTS row-load = ~25-33 us of pure PE time.
  - XBAR DMA transposes (dma_start_transpose): 256B packets cap aggregate
    DMA at ~150 GB/s (per-engine packet-rate limit), starving the PE (596 us).
  - fp8 DoubleRow matmul (0.5 cyc/row): paces at the 130 ns LDWEIGHTS, not
    the 53 ns stream -> 131 ns/mm measured = LOWER MAC rate than bf16.
"""

import os
import sys

for _p in (
    "/opt/trn_rl_repo",
    "/root/.axon_site",
    "/root/.axon_site/_ro/trn_rl_repo",
    "/root/.axon_site/_ro/pypackages",
):
    if os.path.isdir(_p) and _p not in sys.path:
        sys.path.append(_p)

import numpy as np
import ml_dtypes

B, S, DI, DO = 4, 2048, 4096, 4096
NCORES = 8
M = B * S            # 8192
MC = M // NCORES     # 1024 rows per core
P = 128
KT = DI // P         # 32 k-tiles
MT = MC // P         # 8 m-tiles per core
OBW = 512            # o-block width
OB = DO // OBW       # 8 o-blocks
WCK = 16             # k-tiles per weight chunk (2 x 1MB DMAs per o-block:
WCH = KT // WCK      # fewer ~600ns DMA dispatches serializing the launch)

_STATE = {}


def _build_program():
    import concourse.bass as bass
    import concourse.mybir as mybir
    import concourse.tile as tile
    from concourse import bacc

    dt = mybir.dt
    F32, BF16, FP8 = dt.float32, dt.bfloat16, dt.float8e4

    nc = bacc.Bacc(target_bir_lowering=False)

    x_in = nc.dram_tensor("x", [MT, P, KT, P], F32, kind="ExternalInput")
    w_in = nc.dram_tensor("w", [OB, P, KT, OBW], FP8, kind="ExternalInput")
    s_in = nc.dram_tensor("s", [P, 1], F32, kind="ExternalInput")
    b_in = nc.dram_tensor("b", [P, DO], F32, kind="ExternalInput")
    y_out = nc.dram_tensor("y", [MC, DO], F32, kind="ExternalOutput")

    with tile.TileContext(nc) as tc:
        with (
            tc.tile_pool(name="const", bufs=1) as const,
            tc.tile_pool(name="xt_pool", bufs=1) as xt_pool,
            tc.tile_pool(name="xin_pool", bufs=5) as xin_pool,
            tc.tile_pool(name="w8_pool", bufs=6) as w8_pool,
            tc.tile_pool(name="bias_pool", bufs=2) as bias_pool,
            tc.tile_pool(name="out_pool", bufs=2) as out_pool,
            tc.tile_pool(name="mm_ps_pool", bufs=4, space="PSUM") as mm_ps_pool,
        ):
            # resident transposed bf16 x: [d-part, mt, kt, m]
            xt = xt_pool.tile([P, MT, KT, P], BF16)

            def load_w_part(ob, c0, c1):
                wchunks = []
                for c in range(c0, c1):
                    w8c = w8_pool.tile([P, WCK, OBW], FP8, name=f"w8_{ob}_{c}", tag="w8")
                    nc.sync.dma_start(out=w8c, in_=w_in[ob, :, c * WCK:(c + 1) * WCK, :])
                    wchunks.append(w8c)
                return wchunks

            def load_bias(ob):
                # ONE DMA + an on-device duplicate (evictions come much later)
                bias2 = bias_pool.tile([P, 2 * OBW], F32, name=f"bias2_{ob}", tag="bias")
                nc.sync.dma_start(
                    out=bias2[:, 0:OBW], in_=b_in[:, ob * OBW:(ob + 1) * OBW],
                )
                nc.vector.tensor_copy(out=bias2[:, OBW:2 * OBW], in_=bias2[:, 0:OBW])
                return bias2

            def load_wchunks(ob):
                # w chunks first (they gate the matmuls), bias after
                wchunks = load_w_part(ob, 0, WCH)
                return load_bias(ob), wchunks

            HK = KT // 2

            def x_half(mt, h):
                xin = xin_pool.tile([P, HK, P], F32, name=f"xin_{mt}_{h}", tag="xin")
                nc.sync.dma_start(out=xin, in_=x_in[mt, :, h * HK:(h + 1) * HK, :])
                nc.scalar.copy(xt[:, mt, h * HK:(h + 1) * HK, :], xin)

            def x_chain(mt):
                # one m-tile of pre-transposed x, as two INDEPENDENT
                # half-tiles (1MB DMA + ACT bf16 cast each) so the matmuls
                # can start on k-tiles 0..15 while 16..31 still stream
                x_half(mt, 0)
                x_half(mt, 1)

            def emit_group(ps_h, ob, mt, wchunks):
                for kt in range(KT):
                    wb_sl = wchunks[kt // WCK][:, kt % WCK, :]
                    nc.tensor.matmul(
                        ps_h, xt[:, mt, kt, :], wb_sl,
                        start=(kt == 0), stop=(kt == KT - 1),
                        skip_group_check=True,
                    )

            def mm_pair(ob, mt0, bias2, wchunks):
                # two m-tile groups share one 2-bank PSUM tile and a single
                # fused eviction -> half the group-boundary syncs on PE
                ps = mm_ps_pool.tile([P, 2 * OBW], F32, name=f"ps_{ob}_{mt0}", tag="ps")
                for h, mt in ((0, mt0), (1, mt0 + 1)):
                    emit_group(ps[:, h * OBW:(h + 1) * OBW], ob, mt, wchunks)
                out_sb = out_pool.tile([P, 2 * OBW], F32, name=f"o_{ob}_{mt0}", tag="out")
                nc.vector.scalar_tensor_tensor(
                    out_sb, ps, s2[:, :], bias2,
                    mybir.AluOpType.mult, mybir.AluOpType.add,
                )
                for h, mt in ((0, mt0), (1, mt0 + 1)):
                    nc.sync.dma_start(
                        out=y_out[mt * P:(mt + 1) * P, ob * OBW:(ob + 1) * OBW],
                        in_=out_sb[:, h * OBW:(h + 1) * OBW],
                    )

            def mm_single(ob, mt, bias2, wchunks):
                # single-m-tile group: finer granularity at the pipeline edge
                # (same PSUM shape as pairs so the pool keeps one slot kind)
                ps = mm_ps_pool.tile([P, 2 * OBW], F32, name=f"pss_{ob}_{mt}", tag="ps")
                ps = ps[:, 0:OBW]
                emit_group(ps, ob, mt, wchunks)
                out_sb = out_pool.tile([P, OBW], F32, name=f"os_{ob}_{mt}", tag="outs")
                nc.vector.scalar_tensor_tensor(
                    out_sb, ps, s2[:, :], bias2[:, 0:OBW],
                    mybir.AluOpType.mult, mybir.AluOpType.add,
                )
                nc.sync.dma_start(
                    out=y_out[mt * P:(mt + 1) * P, ob * OBW:(ob + 1) * OBW],
                    in_=out_sb,
                )

            # ---- Phase T: pair p's matmuls (o-blocks 0..1) run while pair
            # p+1's x streams in ----
            # DMA-queue-ordered prologue: interleave x(mt0) halves with the
            # first halves of BOTH o-blocks' w streams so every dep of the
            # first two matmul groups lands just-in-time
            x_half(0, 0)
            wch0 = load_w_part(0, 0, 1)
            wch1 = load_w_part(1, 0, 1)
            x_half(0, 1)
            wch0 += load_w_part(0, 1, WCH)
            wch1 += load_w_part(1, 1, WCH)
            # s2 + biases AFTER the matmul-gating loads (needed only by the
            # evictions, ~20us later)
            s_t = const.tile([P, 1], F32)
            nc.sync.dma_start(out=s_t, in_=s_in[:, :])
            s2 = const.tile([P, 1], F32)
            nc.scalar.mul(s2, s_t, 2.0)  # fold back the /2 from the fp8 re-encode
            bias0 = load_bias(0)
            bias1 = load_bias(1)
            bias_w = [(bias0, wch0), (bias1, wch1)]
            mm_single(0, 0, *bias_w[0])
            x_chain(1)
            mm_single(1, 0, *bias_w[1])
            x_chain(2)
            mm_single(0, 1, *bias_w[0])
            x_chain(3)
            mm_single(1, 1, *bias_w[1])
            for mt0 in range(2, MT, 2):
                for ob in (0, 1):
                    mm_pair(ob, mt0, *bias_w[ob])
                if mt0 + 2 < MT:
                    x_chain(mt0 + 2)
                    x_chain(mt0 + 3)

            # ---- Phase B: o-blocks 2..7 stream w fp8 from DRAM against the
            # resident xt ----
            for ob in range(2, OB):
                bias2, wchunks = load_wchunks(ob)
                for mt0 in range(0, MT, 2):
                    mm_pair(ob, mt0, bias2, wchunks)

    nc.finalize()
    return nc


def _get_program():
    if "nc" not in _STATE:
        _STATE["nc"] = _build_program()
    return _STATE["nc"]


def _prep_weights(weight_fp8):
    """Re-encode jax e4m3fn bytes as IEEE-e4m3 bytes of value/2 (exact for
    normals), transpose to [d, o], and block to [ob, p, kt, obw] so each
    o-block DMA reads 2KB-contiguous per-partition lines."""
    bits = np.arange(256, dtype=np.uint8)
    vals = bits.view(ml_dtypes.float8_e4m3fn).astype(np.float32) * 0.5
    lut = vals.astype(ml_dtypes.float8_e4m3).view(np.uint8)

    wb = np.asarray(weight_fp8).view(np.uint8)          # [DO, DI]
    w2t = np.ascontiguousarray(lut[wb].T)               # [DI, DO]
    w_pre = np.ascontiguousarray(
        w2t.reshape(KT, P, OB, OBW).transpose(2, 1, 0, 3)
    )                                                   # [OB, P, KT, OBW]
    return w_pre.view(ml_dtypes.float8_e4m3)


def _prep_x(x_core):
    """Block one core's x [MC, DI] to [MT, P(d), KT, P(m)] — the transposed
    tile layout the matmul stationary wants. Pure layout permutation (no
    arithmetic; the bf16 rounding happens on device)."""
    return np.ascontiguousarray(
        x_core.reshape(MT, P, KT, P).transpose(0, 3, 2, 1)
    )


def kernel(x, weight_fp8, weight_inv_scale, bias):
    from concourse.bass_utils import run_bass_kernel_spmd

    try:
        import jax
        jax.config.update("jax_compilation_cache_dir", "/tmp/jax_neff_cache")
        jax.config.update("jax_persistent_cache_min_entry_size_bytes", 0)
        jax.config.update("jax_persistent_cache_min_compile_time_secs", 0.0)
    except Exception:
        pass

    nc = _get_program()

    x_np = np.asarray(x, dtype=np.float32).reshape(M, DI)
    w_pre = _prep_weights(weight_fp8)
    s_b = np.ascontiguousarray(
        np.broadcast_to(
            np.asarray(weight_inv_scale, dtype=np.float32).reshape(1, 1), (P, 1)
        )
    )
    b_b = np.ascontiguousarray(
        np.broadcast_to(np.asarray(bias, dtype=np.float32), (P, DO))
    )

    core_ids = list(range(NCORES))
    in_maps = [
        {"x": _prep_x(x_np[c * MC:(c + 1) * MC]), "w": w_pre, "s": s_b, "b": b_b}
        for c in core_ids
    ]

    last_err = None
    for _attempt in range(3):
        try:
            res = run_bass_kernel_spmd(nc, in_maps, core_ids)
            break
        except Exception as e:  # device wedge (NRT_EXEC_UNIT_UNRECOVERABLE): reset + retry
            last_err = e
            try:
                import jax
                import time
                jax.clear_backends()
                time.sleep(3.0)
            except Exception:
                pass
    else:
        raise last_err

    y = np.concatenate([res.results[c]["y"] for c in core_ids], axis=0)
    return y.reshape(B, S, DO)


# revision 40
# speedup vs baseline: 1.1932x; 1.1932x over previous
"""TRN2 Bass kernel for nn_FP8LinearWrapper: y = x @ (w_fp8 * inv_scale).T + bias.

Strategy (8 NeuronCores, SPMD):
  - Data-parallel over the flattened token dim: x [4,2048,4096] -> [8192,4096],
    1024 rows per core. Weights/bias replicated to every core.
  - Per core: SINGLE-pass bf16 matmul. x arrives host-blocked to the
    transposed tile layout [MT, P(d), KT, P(m)] (pure layout prep, same
    class as the weight transpose/re-block below - no host arithmetic);
    the device rounds it to bf16 (ACT cast, the only arithmetic on x) into
    a resident SBUF operand. The fp8 weight is fed directly as the matmul
    moving operand (mixed bf16 x fp8e4 matmul, verified bit-exact on HW);
    accumulation in fp32 PSUM. Error is dominated by the bf16 rounding of
    x: ~1.7e-3 rel absmax vs the 2e-2 gate (the dual-pass hi+lo scheme
    this replaces was 2x the PE work for accuracy the gate does not need).
  - The fp8 weight bytes are jax float8_e4m3fn (max 448). TRN2's fp8e4 decode
    is IEEE e4m3 (max 240), so the host re-encodes each byte via a LUT to the
    e4m3 bits of (value/2) - exact for all normals - and the kernel folds the
    missing *2 into the output scale. w is passed pre-transposed/pre-blocked
    (weight layout prep, as for any serving stack).

Timing structure (measured ~472 us/core total; PE runs ONLY the 2048
[128k x 128m x 512o] matmuls at the 216 ns/MM N=512 bf16 streaming roofline
= 442 us/core and is ~99.7% busy within its span; x casts ride the idle ACT
engine, evictions the DVE; launch ~15 us, drain ~13 us):
  - Phase T: m-tile pairs; per pair the PE runs o-blocks 0..1 (128 matmuls)
    while the NEXT pair's x streams in (2MB DMA + 2 half-casts per m-tile).
    x-chains are emitted one pair BEHIND the matmuls; pair 0 runs as
    per-m-tile singles so the first matmul needs only m-tile 0's chunk.
  - Phase B: o-blocks 2..7 stream w fp8 from DRAM against the resident xt.
  - m-tile pairs share one 2-bank PSUM tile (2 x 32 accumulating matmuls)
    with a single fused (psum * 2*inv_scale) + bias DVE eviction covering
    both banks; with no PE-transpose PSUM pool needed, the matmul pool gets
    4 bufs (all 8 banks) so evictions never gate group starts.
Rejected alternatives (measured on HW):
  - On-device PE transposes (prior version, 490 us): 256 transposes pace at
    the ~130 ns LDWEIGHTS row-load = ~25-33 us of pure PE time.
  - XBAR DMA transposes (dma_start_transpose): 256B packets cap aggregate
    DMA at ~150 GB/s (per-engine packet-rate limit), starving the PE (596 us).
  - fp8 DoubleRow matmul (0.5 cyc/row): paces at the 130 ns LDWEIGHTS, not
    the 53 ns stream -> 131 ns/mm measured = LOWER MAC rate than bf16.
"""

import os
import sys

for _p in (
    "/opt/trn_rl_repo",
    "/root/.axon_site",
    "/root/.axon_site/_ro/trn_rl_repo",
    "/root/.axon_site/_ro/pypackages",
):
    if os.path.isdir(_p) and _p not in sys.path:
        sys.path.append(_p)

import numpy as np
import ml_dtypes

B, S, DI, DO = 4, 2048, 4096, 4096
NCORES = 8
M = B * S            # 8192
MC = M // NCORES     # 1024 rows per core
P = 128
KT = DI // P         # 32 k-tiles
MT = MC // P         # 8 m-tiles per core
OBW = 512            # o-block width
OB = DO // OBW       # 8 o-blocks
WCK = 4              # k-tiles per weight chunk
WCH = KT // WCK      # 8 weight chunks per o-block

_STATE = {}


def _build_program():
    import concourse.bass as bass
    import concourse.mybir as mybir
    import concourse.tile as tile
    from concourse import bacc

    dt = mybir.dt
    F32, BF16, FP8 = dt.float32, dt.bfloat16, dt.float8e4

    nc = bacc.Bacc(target_bir_lowering=False)

    x_in = nc.dram_tensor("x", [MT, P, KT, P], F32, kind="ExternalInput")
    w_in = nc.dram_tensor("w", [OB, P, KT, OBW], FP8, kind="ExternalInput")
    s_in = nc.dram_tensor("s", [P, 1], F32, kind="ExternalInput")
    b_in = nc.dram_tensor("b", [P, DO], F32, kind="ExternalInput")
    y_out = nc.dram_tensor("y", [MC, DO], F32, kind="ExternalOutput")

    with tile.TileContext(nc) as tc:
        with (
            tc.tile_pool(name="const", bufs=1) as const,
            tc.tile_pool(name="xt_pool", bufs=1) as xt_pool,
            tc.tile_pool(name="xin_pool", bufs=5) as xin_pool,
            tc.tile_pool(name="w8_pool", bufs=18) as w8_pool,
            tc.tile_pool(name="bias_pool", bufs=2) as bias_pool,
            tc.tile_pool(name="out_pool", bufs=2) as out_pool,
            tc.tile_pool(name="mm_ps_pool", bufs=4, space="PSUM") as mm_ps_pool,
        ):
            # resident transposed bf16 x: [d-part, mt, kt, m]
            xt = xt_pool.tile([P, MT, KT, P], BF16)

            def load_w_part(ob, c0, c1):
                wchunks = []
                for c in range(c0, c1):
                    w8c = w8_pool.tile([P, WCK, OBW], FP8, name=f"w8_{ob}_{c}", tag="w8")
                    nc.sync.dma_start(out=w8c, in_=w_in[ob, :, c * WCK:(c + 1) * WCK, :])
                    wchunks.append(w8c)
                return wchunks

            def load_bias(ob):
                # ONE DMA + an on-device duplicate (evictions come much later)
                bias2 = bias_pool.tile([P, 2 * OBW], F32, name=f"bias2_{ob}", tag="bias")
                nc.sync.dma_start(
                    out=bias2[:, 0:OBW], in_=b_in[:, ob * OBW:(ob + 1) * OBW],
                )
                nc.vector.tensor_copy(out=bias2[:, OBW:2 * OBW], in_=bias2[:, 0:OBW])
                return bias2

            def load_wchunks(ob):
                # w chunks first (they gate the matmuls), bias after
                wchunks = load_w_part(ob, 0, WCH)
                return load_bias(ob), wchunks

            HK = KT // 2

            def x_half(mt, h):
                xin = xin_pool.tile([P, HK, P], F32, name=f"xin_{mt}_{h}", tag="xin")
                nc.sync.dma_start(out=xin, in_=x_in[mt, :, h * HK:(h + 1) * HK, :])
                nc.scalar.copy(xt[:, mt, h * HK:(h + 1) * HK, :], xin)

            def x_chain(mt):
                # one m-tile of pre-transposed x, as two INDEPENDENT
                # half-tiles (1MB DMA + ACT bf16 cast each) so the matmuls
                # can start on k-tiles 0..15 while 16..31 still stream
                x_half(mt, 0)
                x_half(mt, 1)

            def emit_group(ps_h, ob, mt, wchunks):
                for kt in range(KT):
                    wb_sl = wchunks[kt // WCK][:, kt % WCK, :]
                    nc.tensor.matmul(
                        ps_h, xt[:, mt, kt, :], wb_sl,
                        start=(kt == 0), stop=(kt == KT - 1),
                        skip_group_check=True,
                    )

            def mm_pair(ob, mt0, bias2, wchunks):
                # two m-tile groups share one 2-bank PSUM tile and a single
                # fused eviction -> half the group-boundary syncs on PE
                ps = mm_ps_pool.tile([P, 2 * OBW], F32, name=f"ps_{ob}_{mt0}", tag="ps")
                for h, mt in ((0, mt0), (1, mt0 + 1)):
                    emit_group(ps[:, h * OBW:(h + 1) * OBW], ob, mt, wchunks)
                out_sb = out_pool.tile([P, 2 * OBW], F32, name=f"o_{ob}_{mt0}", tag="out")
                nc.vector.scalar_tensor_tensor(
                    out_sb, ps, s2[:, :], bias2,
                    mybir.AluOpType.mult, mybir.AluOpType.add,
                )
                for h, mt in ((0, mt0), (1, mt0 + 1)):
                    nc.sync.dma_start(
                        out=y_out[mt * P:(mt + 1) * P, ob * OBW:(ob + 1) * OBW],
                        in_=out_sb[:, h * OBW:(h + 1) * OBW],
                    )

            def mm_single(ob, mt, bias2, wchunks):
                # single-m-tile group: finer granularity at the pipeline edge
                # (same PSUM shape as pairs so the pool keeps one slot kind)
                ps = mm_ps_pool.tile([P, 2 * OBW], F32, name=f"pss_{ob}_{mt}", tag="ps")
                ps = ps[:, 0:OBW]
                emit_group(ps, ob, mt, wchunks)
                out_sb = out_pool.tile([P, OBW], F32, name=f"os_{ob}_{mt}", tag="outs")
                nc.vector.scalar_tensor_tensor(
                    out_sb, ps, s2[:, :], bias2[:, 0:OBW],
                    mybir.AluOpType.mult, mybir.AluOpType.add,
                )
                nc.sync.dma_start(
                    out=y_out[mt * P:(mt + 1) * P, ob * OBW:(ob + 1) * OBW],
                    in_=out_sb,
                )

            # ---- Phase T: pair p's matmuls (o-blocks 0..1) run while pair
            # p+1's x streams in ----
            # DMA-queue-ordered prologue: interleave x(mt0) halves with the
            # first halves of BOTH o-blocks' w streams so every dep of the
            # first two matmul groups lands just-in-time
            x_half(0, 0)
            wch0 = load_w_part(0, 0, WCH // 2)
            wch1 = load_w_part(1, 0, WCH // 2)
            x_half(0, 1)
            wch0 += load_w_part(0, WCH // 2, WCH)
            wch1 += load_w_part(1, WCH // 2, WCH)
            # s2 + biases AFTER the matmul-gating loads (evictions need them
            # only ~20us later)
            s_t = const.tile([P, 1], F32)
            nc.sync.dma_start(out=s_t, in_=s_in[:, :])
            s2 = const.tile([P, 1], F32)
            nc.scalar.mul(s2, s_t, 2.0)  # fold back the /2 from the fp8 re-encode
            bias0 = load_bias(0)
            bias1 = load_bias(1)
            bias_w = [(bias0, wch0), (bias1, wch1)]
            mm_single(0, 0, *bias_w[0])
            x_chain(1)
            mm_single(1, 0, *bias_w[1])
            x_chain(2)
            mm_single(0, 1, *bias_w[0])
            x_chain(3)
            mm_single(1, 1, *bias_w[1])
            for mt0 in range(2, MT, 2):
                for ob in (0, 1):
                    mm_pair(ob, mt0, *bias_w[ob])
                if mt0 + 2 < MT:
                    x_chain(mt0 + 2)
                    x_chain(mt0 + 3)

            # ---- Phase B: o-blocks 2..7 stream w fp8 from DRAM against the
            # resident xt ----
            for ob in range(2, OB):
                bias2, wchunks = load_wchunks(ob)
                for mt0 in range(0, MT, 2):
                    mm_pair(ob, mt0, bias2, wchunks)

    nc.finalize()
    return nc


def _get_program():
    if "nc" not in _STATE:
        _STATE["nc"] = _build_program()
    return _STATE["nc"]


def _prep_weights(weight_fp8):
    """Re-encode jax e4m3fn bytes as IEEE-e4m3 bytes of value/2 (exact for
    normals), transpose to [d, o], and block to [ob, p, kt, obw] so each
    o-block DMA reads 2KB-contiguous per-partition lines."""
    bits = np.arange(256, dtype=np.uint8)
    vals = bits.view(ml_dtypes.float8_e4m3fn).astype(np.float32) * 0.5
    lut = vals.astype(ml_dtypes.float8_e4m3).view(np.uint8)

    wb = np.asarray(weight_fp8).view(np.uint8)          # [DO, DI]
    w2t = np.ascontiguousarray(lut[wb].T)               # [DI, DO]
    w_pre = np.ascontiguousarray(
        w2t.reshape(KT, P, OB, OBW).transpose(2, 1, 0, 3)
    )                                                   # [OB, P, KT, OBW]
    return w_pre.view(ml_dtypes.float8_e4m3)


def _prep_x(x_core):
    """Block one core's x [MC, DI] to [MT, P(d), KT, P(m)] — the transposed
    tile layout the matmul stationary wants. Pure layout permutation (no
    arithmetic; the bf16 rounding happens on device)."""
    return np.ascontiguousarray(
        x_core.reshape(MT, P, KT, P).transpose(0, 3, 2, 1)
    )


def kernel(x, weight_fp8, weight_inv_scale, bias):
    from concourse.bass_utils import run_bass_kernel_spmd

    try:
        import jax
        jax.config.update("jax_compilation_cache_dir", "/tmp/jax_neff_cache")
        jax.config.update("jax_persistent_cache_min_entry_size_bytes", 0)
        jax.config.update("jax_persistent_cache_min_compile_time_secs", 0.0)
    except Exception:
        pass

    nc = _get_program()

    x_np = np.asarray(x, dtype=np.float32).reshape(M, DI)
    w_pre = _prep_weights(weight_fp8)
    s_b = np.ascontiguousarray(
        np.broadcast_to(
            np.asarray(weight_inv_scale, dtype=np.float32).reshape(1, 1), (P, 1)
        )
    )
    b_b = np.ascontiguousarray(
        np.broadcast_to(np.asarray(bias, dtype=np.float32), (P, DO))
    )

    core_ids = list(range(NCORES))
    in_maps = [
        {"x": _prep_x(x_np[c * MC:(c + 1) * MC]), "w": w_pre, "s": s_b, "b": b_b}
        for c in core_ids
    ]

    last_err = None
    for _attempt in range(3):
        try:
            res = run_bass_kernel_spmd(nc, in_maps, core_ids)
            break
        except Exception as e:  # device wedge (NRT_EXEC_UNIT_UNRECOVERABLE): reset + retry
            last_err = e
            try:
                import jax
                import time
                jax.clear_backends()
                time.sleep(3.0)
            except Exception:
                pass
    else:
        raise last_err

    y = np.concatenate([res.results[c]["y"] for c in core_ids], axis=0)
    return y.reshape(B, S, DO)
